# revision 42
# baseline (speedup 1.0000x reference)
"""Multi-head attention (B=2, S=2048, D=1024, H=16, causal) on 8 TRN2 NeuronCores.

Sharding: core c -> (batch b = c//4, head-group hg = c%4). Each core:
  - projects its batch's query/key/value against a 256-row slice of Wq/Wk/Wv
    (4 heads of 64 dims),
  - runs causal attention for those 4 heads,
  - multiplies by the matching 256-column slice of Wo -> partial [2048, 1024].
Host sums the 4 partials per batch (the tensor-parallel all-reduce) and stacks.

Everything on-device is bf16 (inputs/weights/intermediates; fp32 PSUM
accumulation): halves HBM traffic vs fp32 and runs the PE at full rate for
all matmul widths. rel-err budget 2e-2 >> bf16's ~2e-3.

Attention is computed scores-transposed ([key, query] tiles), exp on ACT with
fused 1/8 scale, causal mask multiply on the diagonal 128x128 chunks, then PV
"flipped": for each 128-query chunk the exp'd score tile is the stationary
operand and V ([token, dim] layout, with a ones column for row-sums) is the
moving operand, accumulating out^T = [query, head_dim+1] in PSUM. That
orientation puts the softmax denominator on the partition axis, so
normalization is a cheap per-partition reciprocal + tensor_scalar_mul
(the fp32 [1, 512] DVE reciprocal in the [dim, query] orientation cost 3.3us
per call). Normalized heads are transposed back to [dim, query] for the
output projection by the DMA crossbar (dma_start_transpose), which keeps the
transposes off the PE entirely.

Scheduling notes (measured on hw):
  - PSUM zero-region rule: start=True clears all PENDING accumulation in the
    2KB bank -> per-qc PV accumulation runs qc-outer (contiguous start..stop).
  - The issuing engine of a DMA stalls in-order on that DMA's data deps, so
    input loads are hoisted ahead of sink DMAs (oT transposes, out writes)
    in the sync hwdge queue; issuing DMAs from ACT/Pool queues measured
    slower.
  - Score tiles are emitted in column-packed pairs spanning a 2-bank PSUM
    tile so one ACT exp covers two key chunks (halves ACT call count).
  - Host pre-blocks x/w layouts so every DMA reads 4-8KB contiguous lines.
  - The tensor engine is duty-cycle throttled (~41us full rate, then
    alternating half rate), so wall time is ~1.5x linear in PE rows.
"""

import sys

for _p in ("/opt/trn_rl_repo", "/root/.axon_site/_ro/trn_rl_repo"):
    if _p not in sys.path:
        sys.path.append(_p)

import numpy as np
import ml_dtypes

import concourse.bacc as bacc
import concourse.tile as tile
import concourse.mybir as mybir
from concourse.bass import MemorySpace
from concourse.bass_utils import run_bass_kernel_spmd

f32 = mybir.dt.float32
bf16 = mybir.dt.bfloat16
Exp = mybir.ActivationFunctionType.Exp

B, S, D, H = 2, 2048, 1024, 16
HD = 64            # head dim
NH = 4             # heads per core
DO = NH * HD       # 256 projection out-dims per core
NCORES = 8
KI = D // 128      # 8 contraction chunks for the projections
QT = 512           # query block
NQT = S // QT      # 4
KT = 128           # key chunk / query sub-chunk
NT = QT // KT      # 4

_cache: dict = {}
PHASE_LOG: list = []

# ablation switches for perf experiments
_opts = {"lead": 2, "mask_engine": "dve", "ob_copy": "dve",
         "psS": 2, "psO": 2, "psA": 2, "xin_bufs": 11, "ptp_bufs": 4}


def _build(repeat: int = 1):
    nc = bacc.Bacc("TRN2", target_bir_lowering=False, debug=False,
                   num_devices=NCORES)

    xqT_d = nc.dram_tensor("xqT", [NQT, 128, KI, QT], bf16,
                           kind="ExternalInput").ap()
    xkT_d = nc.dram_tensor("xkT", [NQT, 128, KI, QT], bf16,
                           kind="ExternalInput").ap()
    xvT_d = nc.dram_tensor("xvT", [NQT, 128, KI, QT], bf16,
                           kind="ExternalInput").ap()
    wqT_d = nc.dram_tensor("wqT", [128, KI, DO], bf16,
                           kind="ExternalInput").ap()
    wkT_d = nc.dram_tensor("wkT", [128, KI, DO], bf16,
                           kind="ExternalInput").ap()
    wvT_d = nc.dram_tensor("wvT", [128, KI, DO], bf16,
                           kind="ExternalInput").ap()
    woT_d = nc.dram_tensor("woT", [128, DO // 128, D], bf16,
                           kind="ExternalInput").ap()
    cmask_d = nc.dram_tensor("cmask", [128, KT], bf16, kind="ExternalInput").ap()
    out_d = nc.dram_tensor("out", [S, D], bf16, kind="ExternalOutput").ap()

    with tile.TileContext(nc) as tc:
        with (
            tc.tile_pool(name="wpool", bufs=1) as wpool,
            tc.tile_pool(name="cpool", bufs=1) as cpool,
            tc.tile_pool(name="persist", bufs=1) as persist,
            tc.tile_pool(name="xin", bufs=_opts["xin_bufs"]) as xin,
            tc.tile_pool(name="ptp", bufs=_opts["ptp_bufs"]) as ptp,
            tc.tile_pool(name="small", bufs=2) as small,
            tc.tile_pool(name="obuf", bufs=2) as obuf,
            tc.tile_pool(name="psS", bufs=_opts["psS"],
                         space=MemorySpace.PSUM) as psS,
            tc.tile_pool(name="psO", bufs=_opts["psO"],
                         space=MemorySpace.PSUM) as psO,
            tc.tile_pool(name="psA", bufs=_opts["psA"],
                         space=MemorySpace.PSUM) as psA,
        ):
            pools = (nc, wpool, cpool, persist, xin, ptp, small, obuf,
                     psS, psO, psA, xqT_d, xkT_d, xvT_d, wqT_d, wkT_d,
                     wvT_d, woT_d, cmask_d, out_d)
            if repeat > 1:
                with tc.For_i(0, repeat):
                    _emit(*pools)
            else:
                _emit(*pools)

    nc.compile()
    return nc


def _emit(nc, wpool, cpool, persist, xin, ptp, small, obuf, psS, psO,
          psA, xqT_d, xkT_d, xvT_d, wqT_d, wkT_d, wvT_d, woT_d, cmask_d,
          out_d):

    # ---- constants / weights ----
    # wq first: the first projection matmul only needs wq + the xq DMA that
    # proj_block(0) issues; the remaining weights stream in behind them.
    wq_sb = wpool.tile([128, KI, DO], bf16, tag="wq")
    nc.sync.dma_start(wq_sb[:], wqT_d)
    wk_sb = wpool.tile([128, KI, DO], bf16, tag="wk")
    wv_sb = wpool.tile([128, KI, DO], bf16, tag="wv")
    wo_sb = wpool.tile([128, DO // 128, D], bf16, tag="wo")
    tri_sb = cpool.tile([128, KT], bf16, tag="tri")

    def qkweights_dma():
        nc.sync.dma_start(wk_sb[:], wkT_d)
        nc.sync.dma_start(wv_sb[:], wvT_d)

    def late_consts_dma():
        # triangular mask tile (j >= i) for the diagonal 128x128 chunks
        nc.sync.dma_start(tri_sb[:], cmask_d)
        nc.sync.dma_start(wo_sb[:], woT_d)

    # ---- per-block persistent intermediates ----
    # qT/kT/oT blocks: [256 dims, QT toks] as [128 parts, 2 chunks, QT]
    #   head j lives in chunk j//2, partitions (j%2)*64 ..+64
    qTt = [persist.tile([128, 2, QT], bf16, tag=f"qT{t}", name=f"qT{t}")
           for t in range(NQT)]
    kTt = [persist.tile([128, 2, QT], bf16, tag=f"kT{t}", name=f"kT{t}")
           for t in range(NQT)]
    oTt = [persist.tile([128, 2, QT], bf16, tag=f"oT{t}", name=f"oT{t}")
           for t in range(NQT)]
    # v blocks, natural layout + ones column: [tokk part, ktc, head, 65]
    vt = [persist.tile([128, NT, NH, HD + 1], bf16, tag=f"v{t}", name=f"v{t}")
          for t in range(NQT)]

    def proj_load(t, after_xq=None, after_xv=None, eng=None):
        """Issue the block's three input DMAs. Emitted EARLY so they sit in
        the sync hwdge queue ahead of the previous attention block's oT
        transposes (the issuing engine stalls in-order on each DMA's data
        dependencies, so sinks behind sources would starve the PE)."""
        eng = eng or nc.sync
        xq = xin.tile([128, KI, QT], bf16, tag="xin", name="xq")
        eng.dma_start(xq[:], xqT_d[t])
        if after_xq is not None:
            after_xq()
        xk = xin.tile([128, KI, QT], bf16, tag="xin", name="xk")
        eng.dma_start(xk[:], xkT_d[t])
        xv = xin.tile([128, KI, QT], bf16, tag="xin", name="xv")
        eng.dma_start(xv[:], xvT_d[t])
        if after_xv is not None:
            after_xv()
        return xq, xk, xv

    def proj_block(t, loads):
        PHASE_LOG.append((f"proj{t}", nc.next_id()))
        xq, xk, xv = loads
        for d in range(2):
            ps = psA.tile([128, QT], f32, tag="ps", name="ps")
            for ki in range(KI):
                nc.tensor.matmul(
                    ps[:], wq_sb[:, ki, d * 128:(d + 1) * 128],
                    xq[:, ki, :], start=(ki == 0), stop=(ki == KI - 1))
            nc.vector.tensor_copy(qTt[t][:, d, :], ps[:])
        for d in range(2):
            ps = psA.tile([128, QT], f32, tag="ps", name="ps")
            for ki in range(KI):
                nc.tensor.matmul(
                    ps[:], wk_sb[:, ki, d * 128:(d + 1) * 128],
                    xk[:, ki, :], start=(ki == 0), stop=(ki == KI - 1))
            nc.vector.tensor_copy(kTt[t][:, d, :], ps[:])
        nc.vector.memset(vt[t][:, :, :, HD], 1.0)
        for tt in range(NT):
            psv = psA.tile([128, DO], f32, tag="ps", name="psv")
            for ki in range(KI):
                nc.tensor.matmul(
                    psv[:], xv[:, ki, tt * KT:(tt + 1) * KT],
                    wv_sb[:, ki, :], start=(ki == 0), stop=(ki == KI - 1))
            nc.vector.tensor_copy(
                vt[t][:, tt, :, 0:HD],
                psv[:].rearrange("p (h e) -> p h e", h=NH))

    def attn_block(qt, fill=None):
        PHASE_LOG.append((f"attn{qt}", nc.next_id()))
        LEAD = _opts["lead"]
        nkt = (qt + 1) * NT
        npair = nkt // 2
        mask_eng = nc.gpsimd if _opts["mask_engine"] == "pool" else nc.vector
        for j in range(NH):
            poff = (j % 2) * HD
            d = j // 2
            qh = qTt[qt][poff:poff + HD, d, :]
            # out^T accumulators: [query part, qc, head_dim + rowsum].
            # PSUM zero-region rule: a start=True matmul clears all PENDING
            # accumulation state in its 2KB bank, so the four qc groups must
            # be emitted as contiguous start..stop runs (qc-outer), never
            # interleaved.
            psov = psO.tile([128, NT, HD + 1], f32, tag="psov", name="psov")
            pts = {}
            emitted = 0

            def emit_score_pair(pi):
                # two key chunks share one 2-bank PSUM tile, column-packed,
                # so one ACT instruction handles both exps
                k0, k1 = 2 * pi, 2 * pi + 1
                pss = psS.tile([128, 2 * QT], f32, tag="pss", name="pss")
                pt = ptp.tile([128, 2 * QT], bf16, tag="pt", name="pt",
                              bufs=NQT * NT // 2 + 2)
                off = 0
                for kt in (k0, k1):
                    r = kt - qt * NT
                    co = max(r, 0) * KT  # column offset into the q block
                    w = QT - co          # restricted width
                    kh = kTt[kt // NT][poff:poff + HD, d,
                                       (kt % NT) * KT:(kt % NT + 1) * KT]
                    nc.tensor.matmul(
                        pss[:, off:off + w], kh, qh[:, co:QT],
                        start=True, stop=True)
                    pts[kt] = (off - co, pt, r, off)
                    off += w
                nc.scalar.activation(pt[:, 0:off], pss[:, 0:off], Exp,
                                     scale=0.125)
                for kt in (k0, k1):
                    _, _, r, o = pts[kt]
                    if r >= 0:
                        mask_eng.tensor_mul(pt[:, o:o + KT], pt[:, o:o + KT],
                                            tri_sb[:])

            for qc in range(NT):
                last_kt = qt * NT + qc
                while emitted * 2 <= min(last_kt + 2 * LEAD, nkt - 1):
                    emit_score_pair(emitted)
                    emitted += 1
                for kt in range(last_kt + 1):
                    base, pt, _, _ = pts[kt]
                    vk = vt[kt // NT][:, kt % NT, j, :]
                    nc.tensor.matmul(
                        psov[:, qc, :], pt[:, base + qc * KT:
                                           base + (qc + 1) * KT], vk,
                        start=(kt == 0), stop=(kt == last_kt))
            pts.clear()
            # normalize: per-partition 1/rowsum, fused into the PSUM->SBUF copy
            rc = small.tile([128, NT], f32, tag="rc", name="rc", bufs=3)
            nc.vector.reciprocal(rc[:], psov[:, :, HD])
            # o_sb shared across the head pair: head j%2 in dim cols
            # poff..poff+64, so one [128,128] DMA transpose covers both heads.
            if j % 2 == 0:
                o_sb = small.tile([128, NT, KT], bf16, tag="osb", name="osb",
                                  bufs=3)
            for qc in range(NT):
                nc.vector.tensor_scalar_mul(
                    o_sb[:, qc, poff:poff + HD], psov[:, qc, 0:HD],
                    rc[:, qc:qc + 1])
            if j % 2 == 1:
                # transpose back to [dim, query] via the DMA crossbar. For
                # the very last pair (all exps done, ACT queue stall-safe)
                # split across both hwdge queues to halve the serial tail.
                last_pair = qt == NQT - 1 and j == NH - 1
                for qc in range(NT):
                    eng = nc.scalar if (last_pair and qc % 2) else nc.sync
                    eng.dma_start_transpose(
                        oTt[qt][:, d, qc * KT:(qc + 1) * KT], o_sb[:, qc, :])
                if fill is not None:
                    fill(j // 2)

    def oproj_block(t, mtts=range(NT)):
        PHASE_LOG.append((f"oproj{t}", nc.next_id()))
        use_act = t == NQT - 1   # ACT is idle after the last exp
        ob_eng = nc.scalar if use_act else {
            "pool": nc.gpsimd, "act": nc.scalar,
            "dve": nc.vector}[_opts["ob_copy"]]
        for mtt in mtts:
            mt = t * NT + mtt
            ob = obuf.tile([128, D], bf16, tag="ob", name="ob")
            for n in range(D // QT):
                ps = psA.tile([128, QT], f32, tag="ps", name="pso2")
                for kc in range(DO // 128):
                    nc.tensor.matmul(
                        ps[:], oTt[t][:, kc, mtt * KT:(mtt + 1) * KT],
                        wo_sb[:, kc, n * QT:(n + 1) * QT],
                        start=(kc == 0), stop=(kc == DO // 128 - 1))
                if use_act or _opts["ob_copy"] == "act":
                    nc.scalar.copy(ob[:, n * QT:(n + 1) * QT], ps[:])
                else:
                    ob_eng.tensor_copy(ob[:, n * QT:(n + 1) * QT], ps[:])
            out_eng = nc.scalar if use_act else nc.sync
            out_eng.dma_start(
                out_d[mt * 128:(mt + 1) * 128, :], ob[:])

    # Block-level software pipeline: each block's projections are emitted one
    # block ahead of its attention so exp never waits on just-finished
    # projections at block boundaries.
    l0 = proj_load(0, after_xq=qkweights_dma, after_xv=late_consts_dma)
    proj_block(0, l0)
    l1 = proj_load(1)
    l2 = proj_load(2)
    proj_block(1, l1)
    l3 = proj_load(3)
    attn_block(0)
    proj_block(2, l2)
    attn_block(1)
    oproj_block(0)
    proj_block(3, l3)
    attn_block(2)
    oproj_block(1)
    attn_block(3, fill=lambda hp: oproj_block(2, mtts=(2 * hp, 2 * hp + 1)))
    oproj_block(3)


def _mask_tile() -> np.ndarray:
    i = np.arange(128)[:, None]
    j = np.arange(KT)[None, :]
    return (j >= i).astype(ml_dtypes.bfloat16)


def _bf(x: np.ndarray) -> np.ndarray:
    return np.ascontiguousarray(np.asarray(x, np.float32).astype(
        ml_dtypes.bfloat16))


def make_in_maps(query, key, value, Wq, Wk, Wv, Wo):
    query = np.asarray(query, np.float32)
    key = np.asarray(key, np.float32)
    value = np.asarray(value, np.float32)
    Wq = np.asarray(Wq, np.float32)
    Wk = np.asarray(Wk, np.float32)
    Wv = np.asarray(Wv, np.float32)
    Wo = np.asarray(Wo, np.float32)
    cm = _mask_tile()

    def xblock(x):
        # [S, D] -> x.T blocked as [NQT, 128, KI, QT] so each per-block DMA
        # reads 8KB-contiguous partition lines
        return _bf(np.ascontiguousarray(
            x.T.reshape(KI, 128, NQT, QT).transpose(2, 1, 0, 3)))

    def wblock(wT):
        # [D_in, n] -> [128, D_in//128, n] contiguous
        return _bf(np.ascontiguousarray(
            wT.reshape(-1, 128, wT.shape[1]).transpose(1, 0, 2)))

    xB = [[xblock(query[b]), xblock(key[b]), xblock(value[b])]
          for b in range(B)]
    in_maps = []
    for c in range(NCORES):
        b, hg = divmod(c, NCORES // B)
        sl = slice(hg * DO, (hg + 1) * DO)
        in_maps.append({
            "xqT": xB[b][0],
            "xkT": xB[b][1],
            "xvT": xB[b][2],
            "wqT": wblock(Wq[sl].T),
            "wkT": wblock(Wk[sl].T),
            "wvT": wblock(Wv[sl].T),
            "woT": wblock(Wo[:, sl].T),
            "cmask": cm,
        })
    return in_maps


def kernel(query, key, value, freqs_complex_form, mask, Wq, Wk, Wv, Wo):
    if "nc" not in _cache:
        _cache["nc"] = _build()
    nc = _cache["nc"]
    in_maps = make_in_maps(query, key, value, Wq, Wk, Wv, Wo)
    res = run_bass_kernel_spmd(nc, in_maps, list(range(NCORES)))
    parts = [np.asarray(res.results[c]["out"], np.float32)
             for c in range(NCORES)]
    npg = NCORES // B
    return np.stack(
        [np.sum(parts[b * npg:(b + 1) * npg], axis=0) for b in range(B)]
    ).astype(np.float32)
